# revision 2
# baseline (speedup 1.0000x reference)
"""Bass/Trainium2 kernel for nn_KernelEdges (gnn_message_passing).

Reference computes A = exp((g_i + g_j - 2*dot_ij)/sigma^2) with zero diag,
broadcast to all B batch slots, where dot is the Gram matrix of
Xf = X.transpose(1,0,2).reshape(N, B*d) and g its diagonal.

Device computes only exp((g_i - 2*dot_ij)/sigma^2) as an [N/8, N] fp16
row-stripe per core; the exact per-column factor exp(g_j/sigma^2), the
zeroed diagonal and the (exact) B-fold batch broadcast are applied on the
host during the gather.  This keeps device HBM traffic at ~2 MB in + 1 MB
out per core.

SPMD trick: the program is identical on all 8 cores, but each core's xt is
column-rotated so its own 256-column block sits at columns 0:256 - the
matmul LHS slice is therefore the same address range on every core, and no
separate lhsT tensor needs to be loaded.  The host un-rotates each stripe
when assembling the output.
"""

import numpy as np

B, N, D = 8, 2048, 64
NCORES = 8
R = N // NCORES          # 256 rows per core
KD = B * D               # 512 contraction dim
NB = 512                 # n-block (one PSUM bank of fp32)
NNB = N // NB            # 4 n-blocks
NMT = R // 128           # 2 m-tiles per core
NQ = KD // 128           # 4 k-tiles

ACT_COLS = 2048          # columns per activation instruction


def _build_program(inv_s2):
    import concourse.bass as bass
    import concourse.tile as tile
    from concourse import bacc, mybir

    f32 = mybir.dt.float32
    f16 = mybir.dt.float16
    bf16 = mybir.dt.bfloat16

    nc = bacc.Bacc(
        "TRN2", target_bir_lowering=False, debug=False, num_devices=NCORES
    )

    xt_d = nc.dram_tensor("xt", [KD, N], bf16, kind="ExternalInput").ap()
    bias_d = nc.dram_tensor("bias", [128, NMT], f32, kind="ExternalInput").ap()
    out_d = nc.dram_tensor("out", [R, N], f16, kind="ExternalOutput").ap()

    with tile.TileContext(nc) as tc:
        with (
            tc.tile_pool(name="persist", bufs=1) as persist,
            tc.tile_pool(name="apool", bufs=1) as apool,
            tc.tile_pool(name="psum", bufs=1, space="PSUM") as pspool,
        ):
            # ---- loads, split across both HWDGE rings ----
            bias_sb = persist.tile([128, NMT], f32, name="bias")
            nc.scalar.dma_start(bias_sb[:], bias_d[:])

            xt_sb = []
            for q in range(NQ):
                t = persist.tile([128, N], bf16, name=f"xt{q}")
                eng = nc.scalar if q < 2 else nc.sync
                eng.dma_start(t[:], xt_d[q * 128:(q + 1) * 128, :])
                xt_sb.append(t)

            # ---- Gram matmuls ----
            # 8 accumulation chains (2 m-tiles x 4 n-blocks) live in the 8
            # PSUM banks at once, grouped as two [128, 2048] tiles so one
            # activation per m-tile can read all four banks.  k-tiles are
            # consumed in load order so the PE overlaps the input DMA; the
            # last round is mt-major so mt0's output pipeline starts first.
            ps = [
                pspool.tile([128, NNB * NB], f32, name=f"ps{mt}")
                for mt in range(NMT)
            ]
            for q in range(NQ):
                for mt in range(NMT):
                    lhs = xt_sb[q][:, mt * 128:(mt + 1) * 128]
                    for nb in range(NNB):
                        nc.tensor.matmul(
                            ps[mt][:, nb * NB:(nb + 1) * NB],
                            lhs,
                            xt_sb[q][:, nb * NB:(nb + 1) * NB],
                            start=(q == 0),
                            stop=(q == NQ - 1),
                        )

            # ---- exp + store ----
            a_sb = [
                apool.tile([128, N], f16, name=f"a{mt}") for mt in range(NMT)
            ]
            for mt in range(NMT):
                for c0 in range(0, N, ACT_COLS):
                    nc.scalar.activation(
                        a_sb[mt][:, c0:c0 + ACT_COLS],
                        ps[mt][:, c0:c0 + ACT_COLS],
                        mybir.ActivationFunctionType.Exp,
                        bias=bias_sb[:, mt:mt + 1],
                        scale=-2.0 * inv_s2,
                    )
            for mt in range(NMT):
                nc.sync.dma_start(
                    out_d[mt * 128:(mt + 1) * 128, :], a_sb[mt][:]
                )

    nc.compile()
    return nc


def _prepare(X, log_sigma):
    """Host prep: returns (inv_s2, in_maps) for run_bass_kernel_spmd."""
    import ml_dtypes

    X = np.ascontiguousarray(X, dtype=np.float32)
    assert X.shape == (B, N, D), X.shape

    sigma = float(np.exp(np.float32(log_sigma)))
    inv_s2 = 1.0 / (sigma * sigma)

    # XT[b*D+f, n] = X[b, n, f]
    XT = np.ascontiguousarray(X.transpose(0, 2, 1).reshape(KD, N))
    g = np.einsum("kn,kn->n", XT, XT).astype(np.float32)  # [N]

    in_maps = []
    for c in range(NCORES):
        r0 = c * R
        # rotate columns so this core's block lands at columns 0:R
        xt_c = np.concatenate([XT[:, r0:], XT[:, :r0]], axis=1)
        bias_np = np.empty((128, NMT), dtype=np.float32)
        for mt in range(NMT):
            bias_np[:, mt] = g[r0 + mt * 128: r0 + (mt + 1) * 128] * inv_s2
        in_maps.append({
            "xt": np.ascontiguousarray(xt_c.astype(ml_dtypes.bfloat16)),
            "bias": bias_np,
        })
    return inv_s2, in_maps


def kernel(X, log_sigma):
    from concourse.bass_utils import run_bass_kernel_spmd

    inv_s2, in_maps = _prepare(X, log_sigma)
    nc = _build_program(inv_s2)
    res = run_bass_kernel_spmd(nc, in_maps, list(range(NCORES)))

    # host-side gather: un-rotate columns, apply the exact per-column
    # exp(g_j/sigma^2) factor, zero the diagonal, broadcast over batch
    Xf = np.ascontiguousarray(X, dtype=np.float32)
    XT = Xf.transpose(0, 2, 1).reshape(KD, N)
    g = np.einsum("kn,kn->n", XT, XT).astype(np.float32)
    colscale = np.exp(g * inv_s2).astype(np.float32)

    A = np.empty((N, N), dtype=np.float32)
    for c in range(NCORES):
        r0 = c * R
        o = np.asarray(res.results[c]["out"]).astype(np.float32)  # [R, N]
        o = np.roll(o, r0, axis=1)
        o *= colscale[None, :]
        A[r0:r0 + R] = o
    idx = np.arange(N)
    A[idx, idx] = 0.0

    out = np.empty((B, N, N), dtype=np.float32)
    out[:] = A[None, :, :]
    return out


# revision 3
# speedup vs baseline: 1.1878x; 1.1878x over previous
"""Bass/Trainium2 kernel for nn_KernelEdges (gnn_message_passing).

Reference computes A = exp((g_i + g_j - 2*dot_ij)/sigma^2) with zero diag,
broadcast to all B batch slots, where dot is the Gram matrix of
Xf = X.transpose(1,0,2).reshape(N, B*d) and g its diagonal.

Device computes only exp((g_i - 2*dot_ij)/sigma^2) as an [N/8, N] fp16
row-stripe per core; the exact per-column factor exp(g_j/sigma^2), the
zeroed diagonal and the (exact) B-fold batch broadcast are applied on the
host during the gather.  This keeps device HBM traffic at ~2 MB in + 1 MB
out per core.

SPMD trick: the program is identical on all 8 cores, but each core's xt is
column-rotated so its own 256-column block sits at columns 0:256 - the
matmul LHS slice is therefore the same address range on every core, and no
separate lhsT tensor needs to be loaded.  The host un-rotates each stripe
when assembling the output.
"""

import numpy as np

B, N, D = 8, 2048, 64
NCORES = 8
R = N // NCORES          # 256 rows per core
KD = B * D               # 512 contraction dim
NB = 512                 # n-block (one PSUM bank of fp32)
NNB = N // NB            # 4 n-blocks
NMT = R // 128           # 2 m-tiles per core
NQ = KD // 128           # 4 k-tiles

ACT_COLS = 2048          # columns per activation instruction


def _build_program(inv_s2):
    import concourse.bass as bass
    import concourse.tile as tile
    from concourse import bacc, mybir

    f32 = mybir.dt.float32
    f16 = mybir.dt.float16
    bf16 = mybir.dt.bfloat16

    nc = bacc.Bacc(
        "TRN2", target_bir_lowering=False, debug=False, num_devices=NCORES
    )

    xt_d = nc.dram_tensor("xt", [KD, N], bf16, kind="ExternalInput").ap()
    bias_d = nc.dram_tensor("bias", [128, NMT], f32, kind="ExternalInput").ap()
    out_d = nc.dram_tensor("out", [R, N], f16, kind="ExternalOutput").ap()

    with tile.TileContext(nc) as tc:
        with (
            tc.tile_pool(name="persist", bufs=1) as persist,
            tc.tile_pool(name="apool", bufs=1) as apool,
            tc.tile_pool(name="psum", bufs=1, space="PSUM") as pspool,
        ):
            # ---- loads ----
            # everything on the sync HWDGE ring: the scalar engine is held
            # up by the exp ACT_TABLE_LOAD early on, and the two rings share
            # the same 16 DMA engines anyway, so a second ring adds latency
            # but no bandwidth.  xt0 first so the PE starts immediately.
            xt_sb = []
            for q in range(NQ):
                t = persist.tile([128, N], bf16, name=f"xt{q}")
                nc.sync.dma_start(t[:], xt_d[q * 128:(q + 1) * 128, :])
                xt_sb.append(t)

            bias_sb = persist.tile([128, NMT], f32, name="bias")
            nc.sync.dma_start(bias_sb[:], bias_d[:])

            # ---- Gram matmuls ----
            # 8 accumulation chains (2 m-tiles x 4 n-blocks) live in the 8
            # PSUM banks at once, grouped as two [128, 2048] tiles so one
            # activation per m-tile can read all four banks.  k-tiles are
            # consumed in load order so the PE overlaps the input DMA; the
            # last round is mt-major so mt0's output pipeline starts first.
            ps = [
                pspool.tile([128, NNB * NB], f32, name=f"ps{mt}")
                for mt in range(NMT)
            ]
            for q in range(NQ):
                for mt in range(NMT):
                    lhs = xt_sb[q][:, mt * 128:(mt + 1) * 128]
                    for nb in range(NNB):
                        nc.tensor.matmul(
                            ps[mt][:, nb * NB:(nb + 1) * NB],
                            lhs,
                            xt_sb[q][:, nb * NB:(nb + 1) * NB],
                            start=(q == 0),
                            stop=(q == NQ - 1),
                        )

            # ---- exp + store ----
            a_sb = [
                apool.tile([128, N], f16, name=f"a{mt}") for mt in range(NMT)
            ]
            for mt in range(NMT):
                for c0 in range(0, N, ACT_COLS):
                    nc.scalar.activation(
                        a_sb[mt][:, c0:c0 + ACT_COLS],
                        ps[mt][:, c0:c0 + ACT_COLS],
                        mybir.ActivationFunctionType.Exp,
                        bias=bias_sb[:, mt:mt + 1],
                        scale=-2.0 * inv_s2,
                    )
            for mt in range(NMT):
                nc.sync.dma_start(
                    out_d[mt * 128:(mt + 1) * 128, :], a_sb[mt][:]
                )

    nc.compile()
    return nc


def _prepare(X, log_sigma):
    """Host prep: returns (inv_s2, in_maps) for run_bass_kernel_spmd."""
    import ml_dtypes

    X = np.ascontiguousarray(X, dtype=np.float32)
    assert X.shape == (B, N, D), X.shape

    sigma = float(np.exp(np.float32(log_sigma)))
    inv_s2 = 1.0 / (sigma * sigma)

    # XT[b*D+f, n] = X[b, n, f]
    XT = np.ascontiguousarray(X.transpose(0, 2, 1).reshape(KD, N))
    g = np.einsum("kn,kn->n", XT, XT).astype(np.float32)  # [N]

    in_maps = []
    for c in range(NCORES):
        r0 = c * R
        # rotate columns so this core's block lands at columns 0:R
        xt_c = np.concatenate([XT[:, r0:], XT[:, :r0]], axis=1)
        bias_np = np.empty((128, NMT), dtype=np.float32)
        for mt in range(NMT):
            bias_np[:, mt] = g[r0 + mt * 128: r0 + (mt + 1) * 128] * inv_s2
        in_maps.append({
            "xt": np.ascontiguousarray(xt_c.astype(ml_dtypes.bfloat16)),
            "bias": bias_np,
        })
    return inv_s2, in_maps


def kernel(X, log_sigma):
    from concourse.bass_utils import run_bass_kernel_spmd

    inv_s2, in_maps = _prepare(X, log_sigma)
    nc = _build_program(inv_s2)
    res = run_bass_kernel_spmd(nc, in_maps, list(range(NCORES)))

    # host-side gather: un-rotate columns, apply the exact per-column
    # exp(g_j/sigma^2) factor, zero the diagonal, broadcast over batch
    Xf = np.ascontiguousarray(X, dtype=np.float32)
    XT = Xf.transpose(0, 2, 1).reshape(KD, N)
    g = np.einsum("kn,kn->n", XT, XT).astype(np.float32)
    colscale = np.exp(g * inv_s2).astype(np.float32)

    A = np.empty((N, N), dtype=np.float32)
    for c in range(NCORES):
        r0 = c * R
        o = np.asarray(res.results[c]["out"]).astype(np.float32)  # [R, N]
        o = np.roll(o, r0, axis=1)
        o *= colscale[None, :]
        A[r0:r0 + R] = o
    idx = np.arange(N)
    A[idx, idx] = 0.0

    out = np.empty((B, N, N), dtype=np.float32)
    out[:] = A[None, :, :]
    return out
